# revision 11
# baseline (speedup 1.0000x reference)
"""MoE combiner kernel for Trainium2 (8 NeuronCores, SPMD).

Computes out[i, d] = sum_e gates[i, e] * expert_outputs[e, d]
  gates:          [16384, 64]  fp32 (top-2 sparse rows, but dense contraction
                                     moves less HBM traffic than a gather)
  expert_outputs: [64, 4096]   fp32
  out:            [16384, 4096] fp32

Sharding: data-parallel over images. Each of the 8 cores computes a
[2048, 4096] slice of the output; the small expert table is replicated.

Math on device: fp32 operands are split host-side into exact fp16
(hi, lo) pairs (hi = fp16(x), lo = fp16(x - hi), after scaling by a power
of two so lo stays in fp16 normal range). The two gate halves are stacked
along the contraction dim (K = 64 experts -> 128 PE rows), so

  psum  = [Ghi; Glo] @ [Ehi; Ehi]   (one K=128 fp16 matmul)
        + [Ghi; Glo] @ [Elo; Elo]   (accumulated, K=128 fp16 matmul)
        = (Ghi + Glo) @ (Ehi + Elo) ~= (G * 2^4) @ (E * 2^8)

and the PSUM->SBUF evacuation rescales by 2^-12. fp16 matmuls stream at
1 column/cycle vs fp32's 4, and the accumulate is fp32 in PSUM, so this
is ~fp32-accurate (~1e-6 rel err) at 4x the PE throughput.
"""

import numpy as np

NUM_EXPERTS = 64
NUM_IMAGES = 16384
D_MODEL = 4096
N_CORES = 8
ROWS = NUM_IMAGES // N_CORES  # 2048 images per core

G_SCALE = 2.0**4   # keeps Glo = fp16(G*16 - fp16(G*16)) in fp16 normal range
E_SCALE = 2.0**8   # same for Elo
OUT_DESCALE = 1.0 / (G_SCALE * E_SCALE)

IMG_TILE = 128          # images per matmul output tile (PSUM partition dim)
N_TILE = 512            # fp32 PSUM bank = 512 floats
GROUP = 1               # image tiles per output DMA (1 -> 2 MiB transfers)
OUT_BUFS = 4

_CACHE = {}


def _build_module():
    import concourse.bacc as bacc
    import concourse.mybir as mybir
    import concourse.tile as tile

    # Bacc (not bare Bass): its compile() pipeline runs
    # move_matmul_waits_to_ldweights + generate_event_semaphores, which
    # legalize multi-sem-wait instructions (the ISA allows one sync wait
    # per instruction; walrus rejects more).
    nc = bacc.Bacc("TRN2")
    f16 = mybir.dt.float16
    f32 = mybir.dt.float32

    n_img_tiles = ROWS // IMG_TILE          # 16
    n_n_tiles = D_MODEL // N_TILE           # 8
    n_groups = n_img_tiles // GROUP         # 8

    with tile.TileContext(nc) as tc:
        with tc.tile_pool(name="dram", bufs=1, space="DRAM") as dram:
            gt = dram.tile([128, ROWS], f16, kind="ExternalInput",
                           name="gt", uniquify=False)
            eh = dram.tile([128, D_MODEL], f16, kind="ExternalInput",
                           name="eh", uniquify=False)
            el = dram.tile([128, D_MODEL], f16, kind="ExternalInput",
                           name="el", uniquify=False)
            out = dram.tile([ROWS, D_MODEL], f32, kind="ExternalOutput",
                            name="out", uniquify=False)
            # out[t*128 + p, d] viewed as [p, t, d] so one DMA can cover
            # GROUP image tiles from a single SBUF tile.
            out_v = out.rearrange("(t p) d -> p t d", p=IMG_TILE)

            with tc.tile_pool(name="const", bufs=1) as cpool, \
                 tc.tile_pool(name="outp", bufs=OUT_BUFS) as outp, \
                 tc.tile_pool(name="psum", bufs=2, space="PSUM") as pspool:
                # Split input loads so compute can start before all input
                # DMA traffic lands.
                gt_sb = cpool.tile([128, ROWS], f16, name="gt_sb")
                for c in range(4):
                    s = slice(c * ROWS // 4, (c + 1) * ROWS // 4)
                    nc.sync.dma_start(out=gt_sb[:, s], in_=gt[:, s])
                eh_sb = cpool.tile([128, D_MODEL], f16, name="eh_sb")
                el_sb = cpool.tile([128, D_MODEL], f16, name="el_sb")
                for c in range(4):
                    s = slice(c * D_MODEL // 4, (c + 1) * D_MODEL // 4)
                    nc.sync.dma_start(out=eh_sb[:, s], in_=eh[:, s])
                    nc.sync.dma_start(out=el_sb[:, s], in_=el[:, s])

                PS_W = 4 * N_TILE  # 4 PSUM banks per evacuation copy
                for it in range(n_img_tiles):
                    ot = outp.tile([128, 1, D_MODEL], f32, name="ot")
                    lhsT = gt_sb[:, it * IMG_TILE:(it + 1) * IMG_TILE]
                    for half in range(D_MODEL // PS_W):
                        ps = pspool.tile([128, PS_W], f32, name="ps")
                        for q in range(PS_W // N_TILE):
                            ns = slice(half * PS_W + q * N_TILE,
                                       half * PS_W + (q + 1) * N_TILE)
                            qs = slice(q * N_TILE, (q + 1) * N_TILE)
                            nc.tensor.matmul(ps[:, qs], lhsT, eh_sb[:, ns],
                                             start=True, stop=False)
                            nc.tensor.matmul(ps[:, qs], lhsT, el_sb[:, ns],
                                             start=False, stop=True)
                        # Rescale while evacuating PSUM; split the copy
                        # load between DVE and ACT.
                        dst = ot[:, 0, half * PS_W:(half + 1) * PS_W]
                        if half % 2 == 0:
                            nc.vector.tensor_scalar_mul(dst, ps[:], OUT_DESCALE)
                        else:
                            nc.scalar.mul(dst, ps[:], OUT_DESCALE)
                    nc.sync.dma_start(
                        out=out_v[:, it:it + 1, :],
                        in_=ot[:])
    nc.compile()
    return nc


def _get_nc():
    if "nc" not in _CACHE:
        _CACHE["nc"] = _build_module()
    return _CACHE["nc"]


def _split_f16(x):
    hi = x.astype(np.float16)
    lo = (x - hi.astype(np.float32)).astype(np.float16)
    return hi, lo


def kernel(expert_outputs: np.ndarray, gates: np.ndarray) -> np.ndarray:
    from concourse.bass_utils import run_bass_kernel_spmd

    nc = _get_nc()

    gs = np.asarray(gates, dtype=np.float32) * np.float32(G_SCALE)
    es = np.asarray(expert_outputs, dtype=np.float32) * np.float32(E_SCALE)
    ghi, glo = _split_f16(gs)
    ehi, elo = _split_f16(es)

    eh_in = np.ascontiguousarray(np.concatenate([ehi, ehi], axis=0))  # [128, D]
    el_in = np.ascontiguousarray(np.concatenate([elo, elo], axis=0))

    in_maps = []
    for c in range(N_CORES):
        rs = slice(c * ROWS, (c + 1) * ROWS)
        gt_c = np.ascontiguousarray(
            np.concatenate([ghi[rs].T, glo[rs].T], axis=0))  # [128, ROWS]
        in_maps.append({"gt": gt_c, "eh": eh_in, "el": el_in})

    res = run_bass_kernel_spmd(nc, in_maps, core_ids=list(range(N_CORES)))
    return np.concatenate([r["out"] for r in res.results], axis=0)


# revision 14
# speedup vs baseline: 1.0704x; 1.0704x over previous
"""MoE combiner kernel for Trainium2 (8 NeuronCores, SPMD).

Computes out[i, d] = sum_e gates[i, e] * expert_outputs[e, d]
  gates:          [16384, 64]  fp32 (top-2 sparse rows, but dense contraction
                                     moves less HBM traffic than a gather)
  expert_outputs: [64, 4096]   fp32
  out:            [16384, 4096] fp32

Sharding: data-parallel over images. Each of the 8 cores computes a
[2048, 4096] slice of the output; the small expert table is replicated.

Math on device: fp32 operands are split host-side into exact fp16
(hi, lo) pairs (hi = fp16(x), lo = fp16(x - hi), after scaling by a power
of two so lo stays in fp16 normal range). The two gate halves are stacked
along the contraction dim (K = 64 experts -> 128 PE rows), so

  psum  = [Ghi; Glo] @ [Ehi; Ehi]   (one K=128 fp16 matmul)
        + [Ghi; Glo] @ [Elo; Elo]   (accumulated, K=128 fp16 matmul)
        = (Ghi + Glo) @ (Ehi + Elo) ~= (G * 2^4) @ (E * 2^8)

and the PSUM->SBUF evacuation rescales by 2^-12. fp16 matmuls stream at
1 column/cycle vs fp32's 4, and the accumulate is fp32 in PSUM, so this
is ~fp32-accurate (~1e-6 rel err) at 4x the PE throughput.
"""

import numpy as np

NUM_EXPERTS = 64
NUM_IMAGES = 16384
D_MODEL = 4096
N_CORES = 8
ROWS = NUM_IMAGES // N_CORES  # 2048 images per core

G_SCALE = 2.0**4   # keeps Glo = fp16(G*16 - fp16(G*16)) in fp16 normal range
E_SCALE = 2.0**8   # same for Elo
OUT_DESCALE = 1.0 / (G_SCALE * E_SCALE)

IMG_TILE = 128          # images per matmul output tile (PSUM partition dim)
N_TILE = 512            # fp32 PSUM bank = 512 floats
GROUP = 1               # image tiles per output DMA (1 -> 2 MiB transfers)
OUT_BUFS = 8

_CACHE = {}


def _build_module():
    import concourse.bacc as bacc
    import concourse.mybir as mybir
    import concourse.tile as tile

    # Bacc (not bare Bass): its compile() pipeline runs
    # move_matmul_waits_to_ldweights + generate_event_semaphores, which
    # legalize multi-sem-wait instructions (the ISA allows one sync wait
    # per instruction; walrus rejects more).
    nc = bacc.Bacc("TRN2")
    f16 = mybir.dt.float16
    f32 = mybir.dt.float32

    n_img_tiles = ROWS // IMG_TILE          # 16
    n_n_tiles = D_MODEL // N_TILE           # 8
    n_groups = n_img_tiles // GROUP         # 8

    with tile.TileContext(nc) as tc:
        with tc.tile_pool(name="dram", bufs=1, space="DRAM") as dram:
            gt = dram.tile([128, ROWS], f16, kind="ExternalInput",
                           name="gt", uniquify=False)
            eh = dram.tile([128, D_MODEL], f16, kind="ExternalInput",
                           name="eh", uniquify=False)
            el = dram.tile([128, D_MODEL], f16, kind="ExternalInput",
                           name="el", uniquify=False)
            out = dram.tile([ROWS, D_MODEL], f32, kind="ExternalOutput",
                            name="out", uniquify=False)
            # out[t*128 + p, d] viewed as [p, t, d] so one DMA can cover
            # GROUP image tiles from a single SBUF tile.
            out_v = out.rearrange("(t p) d -> p t d", p=IMG_TILE)

            with tc.tile_pool(name="const", bufs=1) as cpool, \
                 tc.tile_pool(name="outp", bufs=OUT_BUFS) as outp, \
                 tc.tile_pool(name="psum", bufs=2, space="PSUM") as pspool:
                # One DMA per input: each dma_start costs ~650ns of SP issue
                # time, and the HW ring pipelines the transfers. Order
                # gt -> eh -> el (first matmul needs gt+eh; el only for the
                # accumulate pass).
                gt_sb = cpool.tile([128, ROWS], f16, name="gt_sb")
                nc.sync.dma_start(out=gt_sb[:], in_=gt[:])
                eh_sb = cpool.tile([128, D_MODEL], f16, name="eh_sb")
                nc.sync.dma_start(out=eh_sb[:], in_=eh[:])
                el_sb = cpool.tile([128, D_MODEL], f16, name="el_sb")
                nc.sync.dma_start(out=el_sb[:], in_=el[:])

                PS_W = 4 * N_TILE  # 4 PSUM banks per evacuation copy
                for it in range(n_img_tiles):
                    ot = outp.tile([128, 1, D_MODEL], f32, name="ot")
                    lhsT = gt_sb[:, it * IMG_TILE:(it + 1) * IMG_TILE]
                    for half in range(D_MODEL // PS_W):
                        ps = pspool.tile([128, PS_W], f32, name="ps")
                        # All hi-table matmuls before the lo-table ones so
                        # the first tiles don't stall on the el load.
                        for q in range(PS_W // N_TILE):
                            ns = slice(half * PS_W + q * N_TILE,
                                       half * PS_W + (q + 1) * N_TILE)
                            qs = slice(q * N_TILE, (q + 1) * N_TILE)
                            nc.tensor.matmul(ps[:, qs], lhsT, eh_sb[:, ns],
                                             start=True, stop=False)
                        for q in range(PS_W // N_TILE):
                            ns = slice(half * PS_W + q * N_TILE,
                                       half * PS_W + (q + 1) * N_TILE)
                            qs = slice(q * N_TILE, (q + 1) * N_TILE)
                            nc.tensor.matmul(ps[:, qs], lhsT, el_sb[:, ns],
                                             start=False, stop=True)
                        # Rescale while evacuating PSUM; split the copy
                        # load between DVE and ACT.
                        dst = ot[:, 0, half * PS_W:(half + 1) * PS_W]
                        if half % 2 == 0:
                            nc.vector.tensor_scalar_mul(dst, ps[:], OUT_DESCALE)
                        else:
                            nc.scalar.mul(dst, ps[:], OUT_DESCALE)
                    nc.sync.dma_start(
                        out=out_v[:, it:it + 1, :],
                        in_=ot[:])
    nc.compile()
    return nc


def _get_nc():
    if "nc" not in _CACHE:
        _CACHE["nc"] = _build_module()
    return _CACHE["nc"]


def _split_f16(x):
    hi = x.astype(np.float16)
    lo = (x - hi.astype(np.float32)).astype(np.float16)
    return hi, lo


def kernel(expert_outputs: np.ndarray, gates: np.ndarray) -> np.ndarray:
    from concourse.bass_utils import run_bass_kernel_spmd

    nc = _get_nc()

    gs = np.asarray(gates, dtype=np.float32) * np.float32(G_SCALE)
    es = np.asarray(expert_outputs, dtype=np.float32) * np.float32(E_SCALE)
    ghi, glo = _split_f16(gs)
    ehi, elo = _split_f16(es)

    eh_in = np.ascontiguousarray(np.concatenate([ehi, ehi], axis=0))  # [128, D]
    el_in = np.ascontiguousarray(np.concatenate([elo, elo], axis=0))

    in_maps = []
    for c in range(N_CORES):
        rs = slice(c * ROWS, (c + 1) * ROWS)
        gt_c = np.ascontiguousarray(
            np.concatenate([ghi[rs].T, glo[rs].T], axis=0))  # [128, ROWS]
        in_maps.append({"gt": gt_c, "eh": eh_in, "el": el_in})

    res = run_bass_kernel_spmd(nc, in_maps, core_ids=list(range(N_CORES)))
    return np.concatenate([r["out"] for r in res.results], axis=0)
